# revision 1
# baseline (speedup 1.0000x reference)
"""Multi-head self-attention Trainium2 kernel (B=4, T=2048, C=1024, H=16, D=64).

Sharding: 8 cores = 4 batches x 2 head-groups (8 heads each). Each core
computes its batch's QKV for its heads, attention, and a partial output
projection (row-sharded over attention features). The host sums the two
partials per batch (each partial carries b_proj/2, so the pair sums to
b_proj exactly).

Per-core schedule: V projection first, then per head-pair g: Q/K
projection for g followed by attention for g — so ScalarE exp work
starts early and overlaps the remaining projections on PE.

Device layouts (per core):
  xT   [1024, 2048] bf16 - x[b].T (feature-major tokens)
  wqT/wkT/wvT [1024, 512] bf16 - per-group weight slices, pre-transposed
  bqk  [128, 8] f32      - q/k bias feature-tiles (cols 0-3 q, 4-7 k)
  bvb  [128, 512] f32    - v bias broadcast across partitions
  wpT  [8, 64, 1024] f32r - w_proj slice, per-head chunks, pre-transposed
  bpj  [128, 1024] f32   - b_proj/2 broadcast across partitions
  out: yp [2048, 1024] f32 partial
"""

import numpy as np
import ml_dtypes
from contextlib import ExitStack

import concourse.bass as bass
import concourse.bacc as bacc
import concourse.tile as tile
import concourse.mybir as mybir
from concourse.bass_utils import run_bass_kernel_spmd

F32 = mybir.dt.float32
F32R = mybir.dt.float32r
BF16 = mybir.dt.bfloat16
BF16_NP = ml_dtypes.bfloat16

B, T, C = 4, 2048, 1024
H, D = 16, 64
HL = 8          # heads per core
N_CORES = 8
CC = C // 128   # 8 contraction chunks for QKV
TB = T // 512   # 4 token blocks of 512
TT = T // 128   # 16 token chunks of 128
EXPFN = mybir.ActivationFunctionType.Exp


def build_program():
    nc = bacc.Bacc("TRN2", debug=False, num_devices=1, target_bir_lowering=False)

    xT = nc.dram_tensor("xT", [C, T], BF16, kind="ExternalInput").ap()
    wqT = nc.dram_tensor("wqT", [C, 512], BF16, kind="ExternalInput").ap()
    wkT = nc.dram_tensor("wkT", [C, 512], BF16, kind="ExternalInput").ap()
    wvT = nc.dram_tensor("wvT", [C, 512], BF16, kind="ExternalInput").ap()
    bqk = nc.dram_tensor("bqk", [128, 8], F32, kind="ExternalInput").ap()
    bvb = nc.dram_tensor("bvb", [128, 512], F32, kind="ExternalInput").ap()
    wpT = nc.dram_tensor("wpT", [4, 128, 1024], F32R, kind="ExternalInput").ap()
    bpj = nc.dram_tensor("bpj", [128, 1024], F32, kind="ExternalInput").ap()
    yp = nc.dram_tensor("yp", [T, C], F32, kind="ExternalOutput").ap()

    with tile.TileContext(nc) as tc, ExitStack() as top:
        cpool = top.enter_context(tc.tile_pool(name="consts", bufs=1))
        bqk_sb = cpool.tile([128, 8], F32, tag="bqk")
        nc.sync.dma_start(bqk_sb[:], bqk[:])
        bvb_sb = cpool.tile([128, 512], F32, tag="bvb")
        nc.sync.dma_start(bvb_sb[:], bvb[:])

        actpool = top.enter_context(tc.tile_pool(name="acts", bufs=1))
        OT = {(j, qb): actpool.tile([128, 512], F32R, tag=f"ot{j}_{qb}",
                                    name=f"ot{j}_{qb}")
              for j in range(4) for qb in range(4)}
        QT = {(g, tb): actpool.tile([128, 512], BF16, tag=f"qt{g}_{tb}",
                                    name=f"qt{g}_{tb}")
              for g in range(4) for tb in range(TB)}
        KT = {(g, tb): actpool.tile([128, 512], BF16, tag=f"kt{g}_{tb}",
                                    name=f"kt{g}_{tb}")
              for g in range(4) for tb in range(TB)}
        V = [actpool.tile([128, HL * 65], BF16, tag=f"v{tt}", name=f"v{tt}")
             for tt in range(TT)]

        ps1cm = tc.tile_pool(name="ps1", bufs=2, space="PSUM")
        ps1pool = ps1cm.__enter__()
        attncm = [tc.tile_pool(name="pt", bufs=1),
                  tc.tile_pool(name="ps2", bufs=2, space="PSUM"),
                  tc.tile_pool(name="po", bufs=2, space="PSUM"),
                  tc.tile_pool(name="rr", bufs=3),
                  tc.tile_pool(name="rr0", bufs=3),
                  tc.tile_pool(name="rs", bufs=3),
                  tc.tile_pool(name="otm", bufs=4)]
        (ptpool, ps2pool, popool, rrpool, rr0pool, rspool,
         otmpool) = [cm.__enter__() for cm in attncm]

        xbcm = tc.tile_pool(name="xball", bufs=1)
        xbpool = xbcm.__enter__()

        xb = {}

        def xbv(tb, cc):
            return xb[(tb, cc)][:]

        def load_xb(tb, cc):
            t = xbpool.tile([128, 512], BF16, tag=f"xb{tb}_{cc}",
                            name=f"xb{tb}_{cc}")
            nc.sync.dma_start(
                t[:], xT[cc * 128:(cc + 1) * 128,
                         tb * 512:(tb + 1) * 512])
            xb[(tb, cc)] = t

        # ---- V projection (token-major, ones column per head) ------------
        if True:
            wv_sb = []
            for cc in range(CC):
                t = xbpool.tile([128, 512], BF16, tag=f"wv{cc}", name=f"wv{cc}")
                nc.sync.dma_start(t[:], wvT[cc * 128:(cc + 1) * 128, :])
                wv_sb.append(t)
                load_xb(0, cc)
            for tb in range(1, TB):
                for cc in range(CC):
                    load_xb(tb, cc)
            for tt in range(TT):
                tb, q = tt // 4, tt % 4
                if tt % 2 == 0:
                    ps = ps1pool.tile([128, 512], F32, tag="ps1", name="psv")
                else:
                    ps = popool.tile([128, 512], F32, tag="po", name="psv")
                for cc in range(CC):
                    nc.tensor.matmul(
                        ps[:], xbv(tb, cc)[:, q * 128:(q + 1) * 128],
                        wv_sb[cc][:],
                        start=(cc == 0), stop=(cc == CC - 1))
                v3 = V[tt][:].rearrange("p (h x) -> p h x", x=65)
                nc.gpsimd.memset(v3[:, :, 64:65], 1.0)
                nc.vector.scalar_tensor_tensor(
                    v3[:, :, 0:64],
                    ps[:].rearrange("p (h x) -> p h x", x=64), 1.0,
                    bvb_sb[:].rearrange("p (h x) -> p h x", x=64),
                    op0=mybir.AluOpType.mult, op1=mybir.AluOpType.add)

        # ---- Q/K projection + attention, interleaved per head pair -------
        with tc.tile_pool(name="wqk", bufs=1) as wqkpool:
            wq_all = wqkpool.tile([128, CC * 512], BF16, tag="wq_all")
            nc.sync.dma_start(
                wq_all[:].rearrange("p (c j) -> p c j", j=512),
                wqT.rearrange("(c p) j -> p c j", p=128))
            wk_all = wqkpool.tile([128, CC * 512], BF16, tag="wk_all")
            nc.sync.dma_start(
                wk_all[:].rearrange("p (c j) -> p c j", j=512),
                wkT.rearrange("(c p) j -> p c j", p=128))

            for g in range(4):
                # Q/K projection for this head pair (feature-major)
                for tb in range(TB):
                    for wi, (w_all, dst, bcol) in enumerate(
                            ((wq_all, QT[(g, tb)], g),
                             (wk_all, KT[(g, tb)], 4 + g))):
                        if g == 0 and (2 * tb + wi) % 2 == 1:
                            ps = popool.tile([128, 512], F32, tag="po",
                                             name="psqk")
                        else:
                            ps = ps1pool.tile([128, 512], F32, tag="ps1",
                                              name="psqk")
                        for cc in range(CC):
                            co = cc * 512 + g * 128
                            nc.tensor.matmul(
                                ps[:], w_all[:, co:co + 128],
                                xbv(tb, cc),
                                start=(cc == 0), stop=(cc == CC - 1))
                        nc.vector.tensor_scalar_add(
                            dst[:], ps[:], bqk_sb[:, bcol:bcol + 1])

                # attention for heads 2g, 2g+1
                for qb in range(4):
                    qs = slice(qb * 512, (qb + 1) * 512)
                    pts = [[], []]
                    for kp in range(TT // 2):
                        # alternate row groups so the two heads' score
                        # matmuls can run concurrently in the PE array
                        pp = [ps2pool.tile([128, 1024], F32, tag="ps2",
                                           name=f"sc{j}") for j in range(2)]
                        for j in range(2):
                            kc = 2 * kp + j
                            for j2 in range(2):
                                fo = j2 * 64
                                nc.tensor.matmul(
                                    pp[j2][:, j * 512:(j + 1) * 512],
                                    KT[(g, kc // 4)][fo:fo + 64,
                                                     (kc % 4) * 128:
                                                     (kc % 4 + 1) * 128],
                                    QT[(g, qb)][fo:fo + 64, :],
                                    start=True, stop=True)
                        for j2 in range(2):
                            pt = ptpool.tile([128, 1024], BF16,
                                             tag=f"pt{kp}_{j2}",
                                             name=f"pt{kp}_{j2}")
                            nc.scalar.activation(pt[:], pp[j2][:], EXPFN,
                                                 scale=0.125)
                            pts[j2].append(pt)
                    for j2 in range(2):
                        h = 2 * g + j2
                        po = popool.tile([65, 512], F32, tag="po")
                        for kc in range(TT):
                            nc.tensor.matmul(
                                po[:], V[kc][:, h * 65:(h + 1) * 65],
                                pts[j2][kc // 2][:, (kc % 2) * 512:
                                                 (kc % 2 + 1) * 512],
                                start=(kc == 0), stop=(kc == TT - 1))
                        rr = rrpool.tile([65, 512], F32, tag="rr")
                        nc.vector.reciprocal(rr[64:65, :], po[64:65, :])
                        # partition_broadcast reads the tile's partition 0;
                        # DMA-shift the reciprocal row down first.
                        rr0 = rr0pool.tile([1, 512], F32, tag="rr0")
                        nc.sync.dma_start(rr0[:], rr[64:65, :])
                        rs = rspool.tile([64, 512], F32, tag="rs")
                        nc.gpsimd.partition_broadcast(rs[:], rr0[0:1, :])
                        if j2 == 0:
                            nc.vector.tensor_mul(
                                OT[(g, qb)][0:64, :], po[0:64, :], rs[:])
                        else:
                            otm = otmpool.tile([64, 512], F32R, tag="otm")
                            nc.vector.tensor_mul(otm[:], po[0:64, :], rs[:])
                            nc.sync.dma_start(OT[(g, qb)][64:128, :], otm[:])

        xbcm.__exit__(None, None, None)

        # ---- Output projection (overlaps tail of attention; reuses the
        # ps1 PSUM slots, which are free after the last Q/K group) --------
        with tc.tile_pool(name="wp", bufs=1) as wppool, \
             tc.tile_pool(name="ysb", bufs=4) as ypool:
            wp_all = wppool.tile([128, 4096], F32R, tag="wp_all")
            nc.sync.dma_start(
                wp_all[:].rearrange("p (j o) -> p j o", o=1024),
                wpT.rearrange("j p o -> p j o"))
            bpj_sb = wppool.tile([128, 1024], F32, tag="bpj")
            nc.sync.dma_start(bpj_sb[:], bpj[:])
            for tt in range(TT):
                y_sb = ypool.tile([128, 1024], F32, tag="y")
                for cb in range(2):
                    ps = ps1pool.tile([128, 512], F32, tag="ps1")
                    for j in range(4):
                        nc.tensor.matmul(
                            ps[:],
                            OT[(j, tt // 4)][:, (tt % 4) * 128:
                                             (tt % 4 + 1) * 128],
                            wp_all[:, j * 1024 + cb * 512:
                                   j * 1024 + (cb + 1) * 512],
                            start=(j == 0), stop=(j == 3))
                    nc.vector.tensor_add(
                        y_sb[:, cb * 512:(cb + 1) * 512], ps[:],
                        bpj_sb[:, cb * 512:(cb + 1) * 512])
                nc.sync.dma_start(yp[tt * 128:(tt + 1) * 128, :], y_sb[:])

        for cm in reversed(attncm):
            cm.__exit__(None, None, None)
        ps1cm.__exit__(None, None, None)

    nc.compile()
    return nc


_NC_CACHE = None


def get_program():
    global _NC_CACHE
    if _NC_CACHE is None:
        _NC_CACHE = build_program()
    return _NC_CACHE


def make_in_maps(x, w_qkv, b_qkv, w_proj, b_proj):
    x = np.asarray(x, dtype=np.float32)
    w_qkv = np.asarray(w_qkv, dtype=np.float32)
    b_qkv = np.asarray(b_qkv, dtype=np.float32)
    w_proj = np.asarray(w_proj, dtype=np.float32)
    b_proj = np.asarray(b_proj, dtype=np.float32)

    xTs = [np.ascontiguousarray(x[b].T).astype(BF16_NP) for b in range(B)]
    bpj = np.tile((b_proj * 0.5)[None, :], (128, 1)).astype(np.float32)

    grp = []
    for hg in range(2):
        sl = slice(hg * 512, (hg + 1) * 512)
        wq = w_qkv[0:C][sl]
        wk = w_qkv[C:2 * C][sl]
        wv = w_qkv[2 * C:3 * C][sl]
        bq = b_qkv[0:C][sl]
        bk = b_qkv[C:2 * C][sl]
        bv = b_qkv[2 * C:3 * C][sl]
        grp.append(dict(
            wqT=np.ascontiguousarray(wq.T).astype(BF16_NP),
            wkT=np.ascontiguousarray(wk.T).astype(BF16_NP),
            wvT=np.ascontiguousarray(wv.T).astype(BF16_NP),
            bqk=np.stack([bq[i * 128:(i + 1) * 128] for i in range(4)]
                         + [bk[i * 128:(i + 1) * 128] for i in range(4)],
                         axis=1).astype(np.float32),
            bvb=np.tile(bv[None, :], (128, 1)).astype(np.float32),
            wpT=np.ascontiguousarray(
                w_proj[:, sl].T).reshape(4, 128, 1024),
            bpj=bpj,
        ))

    in_maps = []
    for core in range(N_CORES):
        b, hg = core // 2, core % 2
        m = {"xT": xTs[b]}
        m.update(grp[hg])
        in_maps.append(m)
    return in_maps


def kernel(x, w_qkv, b_qkv, w_proj, b_proj):
    nc = get_program()
    in_maps = make_in_maps(x, w_qkv, b_qkv, w_proj, b_proj)
    res = run_bass_kernel_spmd(
        nc, in_maps, core_ids=list(range(N_CORES)), trace=False)
    y = np.empty((B, T, C), dtype=np.float32)
    for b in range(B):
        y[b] = res.results[2 * b]["yp"] + res.results[2 * b + 1]["yp"]
    return y



# revision 38
# speedup vs baseline: 1.0851x; 1.0851x over previous
"""Multi-head self-attention Trainium2 kernel (B=4, T=2048, C=1024, H=16, D=64).

Sharding: 8 cores = 4 batches x 2 head-groups (8 heads each). Each core
computes its batch's QKV for its heads, attention, and a partial output
projection (row-sharded over attention features). The host sums the two
partials per batch (each partial carries b_proj/2, so the pair sums to
b_proj exactly).

v2 optimizations over the baseline:
  - Scores run as fp8e4 DoubleRow matmuls (0.5 cycles/row): Q/K are
    quantized to fp8 by the bias-add, stored [128, 2, 2048] with the
    second k-subtile zeroed (D=64 < 128, so the pair is (dims, zeros)).
  - P@V runs transposed: out[tok, dim] with P as stationary
    ([128 keys, 128 tok] slices) and V[keys, 65] as moving (ones column
    produces the softmax denominator), so each matmul moves 65 elements
    instead of 512.
  - Softmax normalization becomes a per-partition scalar multiply, then
    PE transposes restore [feat, tok] tiles for the output projection.
  - exp is split across ScalarE (native) and DVE/Pool (bitwise fast-exp
    into bf16 bits).
"""

import numpy as np
import ml_dtypes
from contextlib import ExitStack

import concourse.bass as bass
import concourse.bacc as bacc
import concourse.tile as tile
import concourse.mybir as mybir
from concourse.bass_utils import run_bass_kernel_spmd

F32 = mybir.dt.float32
BF16 = mybir.dt.bfloat16
FP8 = mybir.dt.float8e4
I16 = mybir.dt.int16
BF16_NP = ml_dtypes.bfloat16

B, T, C = 4, 2048, 1024
H, D = 16, 64
HL = 8          # heads per core
N_CORES = 8
CC = C // 128   # 8 contraction chunks for QKV
TB = T // 512   # 4 token blocks of 512
TT = T // 128   # 16 token chunks of 128
EXPFN = mybir.ActivationFunctionType.Exp
DR = mybir.MatmulPerfMode.DoubleRow

USE_FP8_QK = True

# Schraudolph-style exp in bf16 bit space: i16 = x*SCHR_A + SCHR_B, then
# reinterpret the int16 as bf16.  SCHR_A folds the 1/sqrt(D) score scale.
SCHR_A = 0.125 * 128.0 / float(np.log(2.0))
SCHR_B = 127.0 * 128.0 - 7.41

# exp engine per (kp, j2) slot (16 tiles of [128,1024] per block):
# 'A' = ScalarE native exp, 'V' = DVE bitwise fast-exp.  GPSIMD cannot
# read PSUM, so only these two engines can consume score tiles.
EXP_ENG = ['A', 'V', 'A', 'V', 'A', 'V', 'A', 'V',
           'A', 'V', 'A', 'V', 'A', 'V', 'A', 'A']


def build_program():
    nc = bacc.Bacc("TRN2", debug=False, num_devices=1, target_bir_lowering=False)

    xT = nc.dram_tensor("xT", [C, T], BF16, kind="ExternalInput").ap()
    wqT = nc.dram_tensor("wqT", [C, 512], BF16, kind="ExternalInput").ap()
    wkT = nc.dram_tensor("wkT", [C, 512], BF16, kind="ExternalInput").ap()
    wvT = nc.dram_tensor("wvT", [C, 512], BF16, kind="ExternalInput").ap()
    bqk = nc.dram_tensor("bqk", [128, 8], F32, kind="ExternalInput").ap()
    bvb = nc.dram_tensor("bvb", [128, 512], F32, kind="ExternalInput").ap()
    wpT = nc.dram_tensor("wpT", [4, 128, 1024], BF16, kind="ExternalInput").ap()
    bpj = nc.dram_tensor("bpj", [128, 1024], F32, kind="ExternalInput").ap()
    ident = nc.dram_tensor("ident", [128, 128], BF16, kind="ExternalInput").ap()
    yp = nc.dram_tensor("yp", [T, C], F32, kind="ExternalOutput").ap()

    qk_dt = FP8 if USE_FP8_QK else BF16

    with tile.TileContext(nc) as tc, ExitStack() as top:
        cpool = top.enter_context(tc.tile_pool(name="consts", bufs=1))
        bqk_sb = cpool.tile([128, 8], F32, tag="bqk")
        bvb_sb = cpool.tile([128, 512], F32, tag="bvb")
        id_sb = cpool.tile([128, 128], BF16, tag="ident")

        actpool = top.enter_context(tc.tile_pool(name="acts", bufs=1))
        # OT: attention output, feature-major [feat 128, tok 512] bf16
        OT = {(g, qb): actpool.tile([128, 512], BF16, tag=f"ot{g}_{qb}",
                                    name=f"ot{g}_{qb}")
              for g in range(4) for qb in range(4)}
        # Q/K in DoubleRow layout: [128 feat(2 heads), 2 k-subtiles, 2048 tok]
        # subtile 1 is zeros (fp8) / unused (bf16).
        QDR = {g: actpool.tile([128, 2, T], qk_dt, tag=f"qdr{g}",
                               name=f"qdr{g}") for g in range(4)}
        KDR = {g: actpool.tile([128, 2, T], qk_dt, tag=f"kdr{g}",
                               name=f"kdr{g}") for g in range(4)}
        V = [actpool.tile([128, HL * 65], BF16, tag=f"v{tt}", name=f"v{tt}")
             for tt in range(TT)]

        if USE_FP8_QK:
            # zero the second k-subtile once (matmul contracts over both)
            for g in range(4):
                nc.gpsimd.memset(QDR[g][:, 1, :], 0.0)
                nc.gpsimd.memset(KDR[g][:, 1, :], 0.0)

        ps1cm = tc.tile_pool(name="ps1", bufs=2, space="PSUM")
        ps1pool = ps1cm.__enter__()
        attncm = [tc.tile_pool(name="pt", bufs=1),
                  tc.tile_pool(name="ps2", bufs=2, space="PSUM"),
                  tc.tile_pool(name="pvtr", bufs=2, space="PSUM"),
                  tc.tile_pool(name="rr", bufs=2),
                  tc.tile_pool(name="otm", bufs=2)]
        (ptpool, ps2pool, pvpool, rrpool, otmpool) = \
            [cm.__enter__() for cm in attncm]

        xbcm = tc.tile_pool(name="xball", bufs=1)
        xbpool = xbcm.__enter__()

        # x feature-major, all of it: [128, cc, tok]; loaded in 4 big DMAs
        xar = xbpool.tile([128, CC, T], BF16, tag="xar")
        xTr = xT.rearrange("(c p) t -> p c t", p=128)
        wq_all = xbpool.tile([128, CC * 512], BF16, tag="wq_all")
        nc.sync.dma_start(
            wq_all[:].rearrange("p (c j) -> p c j", j=512),
            wqT.rearrange("(c p) j -> p c j", p=128))
        nc.sync.dma_start(xar[:, :, 0:512], xTr[:, :, 0:512])
        nc.sync.dma_start(bqk_sb[:], bqk[:])
        wk_all = xbpool.tile([128, CC * 512], BF16, tag="wk_all")
        nc.sync.dma_start(
            wk_all[:].rearrange("p (c j) -> p c j", j=512),
            wkT.rearrange("(c p) j -> p c j", p=128))
        for tb in range(1, TB):
            nc.sync.dma_start(xar[:, :, tb * 512:(tb + 1) * 512],
                              xTr[:, :, tb * 512:(tb + 1) * 512])
        wv_all = xbpool.tile([128, CC, 512], BF16, tag="wv_all")
        nc.sync.dma_start(wv_all[:],
                          wvT.rearrange("(c p) j -> p c j", p=128))
        nc.sync.dma_start(bvb_sb[:], bvb[:])
        nc.sync.dma_start(id_sb[:], ident[:])

        def xb(tb, cc):
            return xar[:, cc, tb * 512:(tb + 1) * 512]

        def qk_proj_piece(g, tb, which):
            """One Q or K projection group for head pair g, token block tb.
            The bias add runs on ScalarE (per-partition bias AP is legal
            there) and quantizes to fp8 on the way out."""
            w_all, dst, bcol = ((wq_all, QDR[g], g) if which == 0 else
                                (wk_all, KDR[g], 4 + g))
            ps = ps1pool.tile([128, 512], F32, tag="ps1", name="psqk")
            for cc in range(CC):
                co = cc * 512 + g * 128
                nc.tensor.matmul(
                    ps[:], w_all[:, co:co + 128],
                    xb(tb, cc),
                    start=(cc == 0), stop=(cc == CC - 1))
            nc.scalar.activation(
                dst[:, 0, tb * 512:(tb + 1) * 512], ps[:],
                mybir.ActivationFunctionType.Identity,
                bias=bqk_sb[:, bcol:bcol + 1])

        def qk_proj(g):
            for tb in range(TB):
                for which in range(2):
                    qk_proj_piece(g, tb, which)

        def v_proj_piece(tt):
            ps = ps1pool.tile([128, 512], F32, tag="ps1", name="psv")
            for cc in range(CC):
                nc.tensor.matmul(
                    ps[:], xar[:, cc, tt * 128:(tt + 1) * 128],
                    wv_all[:, cc, :],
                    start=(cc == 0), stop=(cc == CC - 1))
            v3 = V[tt][:].rearrange("p (h x) -> p h x", x=65)
            nc.gpsimd.memset(v3[:, :, 64:65], 1.0)
            nc.vector.scalar_tensor_tensor(
                v3[:, :, 0:64],
                ps[:].rearrange("p (h x) -> p h x", x=64), 1.0,
                bvb_sb[:].rearrange("p (h x) -> p h x", x=64),
                op0=mybir.AluOpType.mult, op1=mybir.AluOpType.add)

        # ---- attention blocks, software-pipelined -----------------------
        # block n = (g, qb), qb-major so every fourth block finishes an
        # OT column and the output projection can interleave early.
        # scores(n) and PV(n-1) interleave in the PE stream so PE has
        # work while exp drains score PSUMs.
        blocks = [(g, qb) for qb in range(4) for g in range(4)]
        pts = {}      # (parity, kp, j2) -> exp'd score tile [128, 1024]
        pv_state = {}  # live PV psum tiles per j2

        def scores_mm(n, g, qb, kp, j2):
            """Scores for head 2g+j2, key chunks 2kp/2kp+1, queries qb."""
            pp = ps2pool.tile([128, 1024], F32, tag="ps2", name=f"sc{j2}")
            fo = 64 * j2
            for j in range(2):
                kc = 2 * kp + j
                if USE_FP8_QK:
                    for u in range(2):
                        nc.tensor.matmul(
                            pp[:, j * 512 + u * 256:j * 512 + (u + 1) * 256],
                            KDR[g][fo:fo + 64, :, kc * 128:(kc + 1) * 128],
                            QDR[g][fo:fo + 64, :,
                                   qb * 512 + u * 256:qb * 512 + (u + 1) * 256],
                            start=True, stop=True, perf_mode=DR)
                else:
                    nc.tensor.matmul(
                        pp[:, j * 512:(j + 1) * 512],
                        KDR[g][fo:fo + 64, 0, kc * 128:(kc + 1) * 128],
                        QDR[g][fo:fo + 64, 0, qb * 512:(qb + 1) * 512],
                        start=True, stop=True)
            # exp -> pt tile (bf16), engine per EXP_ENG slot
            pt = ptpool.tile([128, 1024], BF16, tag=f"pt{n % 2}_{kp}_{j2}",
                             name=f"pt{n % 2}_{kp}_{j2}")
            if EXP_ENG[2 * kp + j2] == 'A':
                nc.scalar.activation(pt[:], pp[:], EXPFN, scale=0.125)
            else:
                nc.vector.tensor_scalar(pt[:].bitcast(I16), pp[:],
                                        SCHR_A, SCHR_B,
                                        op0=mybir.AluOpType.mult,
                                        op1=mybir.AluOpType.add)
            pts[(n % 2, kp, j2)] = pt

        # PV accumulation order per head half: groups ts0..ts3, 16 key
        # chunks each, strictly sequential (one open accumulation group
        # per PSUM bank).  Spread over steps 0..5 so the finish work can
        # run at steps 6-7 without delaying the next block's exps.
        PV_SPLIT = [0, 11, 22, 33, 44, 54, 64]

        def pv_mm(n, g, qb, step):
            par = n % 2
            for j2 in range(2):
                h = 2 * g + j2
                if step == 0:
                    pv_state[j2] = pvpool.tile([128, 260], F32,
                                               tag="pvtr", name=f"pv{j2}")
                pv = pv_state[j2]
                for i in range(PV_SPLIT[step], PV_SPLIT[step + 1]):
                    ts, kc = i // 16, i % 16
                    nc.tensor.matmul(
                        pv[:, ts * 65:(ts + 1) * 65],
                        pts[(par, kc // 2, j2)][:, (kc % 2) * 512 + ts * 128:
                                                (kc % 2) * 512 + (ts + 1) * 128],
                        V[kc][:, h * 65:(h + 1) * 65],
                        start=(kc == 0), stop=(kc == 15))

        def pv_finish(n, g, qb):
            """Normalize, transpose and store OT tiles for block n."""
            otm = {}
            for j2 in range(2):
                pv = pv_state.pop(j2)
                rr = rrpool.tile([128, 4], F32, tag="rr", name="rr")
                pv3 = pv[:].rearrange("p (t x) -> p t x", x=65)
                nc.vector.reciprocal(rr[:], pv3[:, :, 64])
                ot = otmpool.tile([128, 256], BF16, tag=f"otm{j2}",
                                  name=f"otm{j2}")
                nc.vector.tensor_tensor(
                    ot[:].rearrange("p (t x) -> p t x", x=64),
                    pv3[:, :, 0:64],
                    rr[:].unsqueeze(-1).broadcast_to([128, 4, 64]),
                    op=mybir.AluOpType.mult)
                otm[j2] = ot
            for ts in range(4):
                tr = ps1pool.tile([128, 128], BF16, tag="ps1", name="tr")
                for j2 in range(2):
                    nc.tensor.matmul(
                        tr[64 * j2:64 * j2 + 64, :],
                        otm[j2][:, ts * 64:(ts + 1) * 64],
                        id_sb[:], start=True, stop=True, is_transpose=True)
                nc.scalar.copy(
                    OT[(g, qb)][:, ts * 128:(ts + 1) * 128], tr[:])

        # ---- output projection piece (interleaved into late blocks) ----
        wp_state = {}

        def outproj_load():
            wpcm = tc.tile_pool(name="wp", bufs=1)
            ycm = tc.tile_pool(name="ysb", bufs=2)
            wp_state["cms"] = [wpcm, ycm]
            wppool = wpcm.__enter__()
            ypool = ycm.__enter__()
            wp_all = wppool.tile([128, 4096], BF16, tag="wp_all")
            nc.sync.dma_start(
                wp_all[:].rearrange("p (j o) -> p j o", o=1024),
                wpT.rearrange("j p o -> p j o"))
            bpj_sb = wppool.tile([128, 1024], F32, tag="bpj")
            nc.sync.dma_start(bpj_sb[:], bpj[:])
            wp_state.update(wp_all=wp_all, bpj_sb=bpj_sb, ypool=ypool)

        def outproj_piece(tt):
            wp_all, bpj_sb = wp_state["wp_all"], wp_state["bpj_sb"]
            y_sb = wp_state["ypool"].tile([128, 1024], F32, tag="y",
                                          name="y_sb")
            for cb in range(2):
                ps = ps1pool.tile([128, 512], F32, tag="ps1", name="psy")
                for j in range(4):
                    nc.tensor.matmul(
                        ps[:],
                        OT[(j, tt // 4)][:, (tt % 4) * 128:
                                         (tt % 4 + 1) * 128],
                        wp_all[:, j * 1024 + cb * 512:
                               j * 1024 + (cb + 1) * 512],
                        start=(j == 0), stop=(j == 3))
                nc.vector.tensor_add(
                    y_sb[:, cb * 512:(cb + 1) * 512], ps[:],
                    bpj_sb[:, cb * 512:(cb + 1) * 512])
            nc.sync.dma_start(yp[tt * 128:(tt + 1) * 128, :], y_sb[:])

        # ---- emit ------------------------------------------------------
        # outproj tile tt is ready once OT[(3, tt//4)] exists, i.e. after
        # pv_finish(block 4*(tt//4)+3) which is emitted during block
        # 4*(tt//4)+4; schedule two tiles per block starting one later.
        outmap = {5: [0, 1], 6: [2, 3], 9: [4, 5], 10: [6, 7],
                  13: [8, 9], 14: [10, 11]}
        qk_proj(0)
        for n, (g, qb) in enumerate(blocks):
            if n == 4:
                # x / qkv-weight tiles are dead; reuse their SBUF for the
                # output projection weights
                xbcm.__exit__(None, None, None)
                outproj_load()
            extra = []
            if n == 0:
                extra = [(v_proj_piece, (tt,)) for tt in range(TT)]
            if n < 3:
                extra += [(qk_proj_piece, (n + 1, tb, w))
                          for tb in range(TB) for w in range(2)]
            extra += [(outproj_piece, (tt,)) for tt in outmap.get(n, [])]
            npc = (len(extra) + 7) // 8  # extra pieces per step
            for kp in range(8):
                if n > 0:
                    pv_mm(n - 1, *blocks[n - 1], step=kp)
                if n > 0 and kp == 7:
                    pv_finish(n - 1, *blocks[n - 1])
                for j2 in range(2):
                    scores_mm(n, g, qb, kp, j2)
                for fn, args in extra[kp * npc:(kp + 1) * npc]:
                    fn(*args)
        for kp in range(8):
            pv_mm(15, *blocks[15], step=kp)
        pv_finish(15, *blocks[15])
        for tt in range(12, 16):
            outproj_piece(tt)

        for cm in reversed(wp_state["cms"]):
            cm.__exit__(None, None, None)
        for cm in reversed(attncm):
            cm.__exit__(None, None, None)
        ps1cm.__exit__(None, None, None)

    nc.compile()
    return nc


_NC_CACHE = None


def get_program():
    global _NC_CACHE
    if _NC_CACHE is None:
        _NC_CACHE = build_program()
    return _NC_CACHE


def make_in_maps(x, w_qkv, b_qkv, w_proj, b_proj):
    x = np.asarray(x, dtype=np.float32)
    w_qkv = np.asarray(w_qkv, dtype=np.float32)
    b_qkv = np.asarray(b_qkv, dtype=np.float32)
    w_proj = np.asarray(w_proj, dtype=np.float32)
    b_proj = np.asarray(b_proj, dtype=np.float32)

    xTs = [np.ascontiguousarray(x[b].T).astype(BF16_NP) for b in range(B)]
    bpj = np.tile((b_proj * 0.5)[None, :], (128, 1)).astype(np.float32)
    ident = np.eye(128, dtype=np.float32).astype(BF16_NP)

    grp = []
    for hg in range(2):
        sl = slice(hg * 512, (hg + 1) * 512)
        wq = w_qkv[0:C][sl]
        wk = w_qkv[C:2 * C][sl]
        wv = w_qkv[2 * C:3 * C][sl]
        bq = b_qkv[0:C][sl]
        bk = b_qkv[C:2 * C][sl]
        bv = b_qkv[2 * C:3 * C][sl]
        grp.append(dict(
            wqT=np.ascontiguousarray(wq.T).astype(BF16_NP),
            wkT=np.ascontiguousarray(wk.T).astype(BF16_NP),
            wvT=np.ascontiguousarray(wv.T).astype(BF16_NP),
            bqk=np.stack([bq[i * 128:(i + 1) * 128] for i in range(4)]
                         + [bk[i * 128:(i + 1) * 128] for i in range(4)],
                         axis=1).astype(np.float32),
            bvb=np.tile(bv[None, :], (128, 1)).astype(np.float32),
            wpT=np.ascontiguousarray(
                w_proj[:, sl].T).reshape(4, 128, 1024).astype(BF16_NP),
            bpj=bpj,
            ident=ident,
        ))

    in_maps = []
    for core in range(N_CORES):
        b, hg = core // 2, core % 2
        m = {"xT": xTs[b]}
        m.update(grp[hg])
        in_maps.append(m)
    return in_maps


def kernel(x, w_qkv, b_qkv, w_proj, b_proj):
    nc = get_program()
    in_maps = make_in_maps(x, w_qkv, b_qkv, w_proj, b_proj)
    res = run_bass_kernel_spmd(
        nc, in_maps, core_ids=list(range(N_CORES)), trace=False)
    y = np.empty((B, T, C), dtype=np.float32)
    for b in range(B):
        y[b] = res.results[2 * b]["yp"] + res.results[2 * b + 1]["yp"]
    return y


# revision 65
# speedup vs baseline: 1.1790x; 1.0866x over previous
"""Multi-head self-attention Trainium2 kernel (B=4, T=2048, C=1024, H=16, D=64).

Sharding: 8 cores = 4 batches x 2 head-groups (8 heads each). Each core
computes its batch's QKV for its heads, attention, and a partial output
projection (row-sharded over attention features). The host sums the two
partials per batch (each partial carries b_proj/2, so the pair sums to
b_proj exactly).

v2 optimizations over the baseline:
  - Scores run as fp8e4 DoubleRow matmuls (0.5 cycles/row): Q/K are
    quantized to fp8 by the bias-add, stored [128, 2, 2048] with the
    second k-subtile zeroed (D=64 < 128, so the pair is (dims, zeros)).
  - P@V runs transposed: out[tok, dim] with P as stationary
    ([128 keys, 128 tok] slices) and V[keys, 65] as moving (ones column
    produces the softmax denominator), so each matmul moves 65 elements
    instead of 512.
  - Softmax normalization becomes a per-partition scalar multiply, then
    PE transposes restore [feat, tok] tiles for the output projection.
  - exp is split across ScalarE (native) and DVE/Pool (bitwise fast-exp
    into bf16 bits).
"""

import numpy as np
import ml_dtypes
from contextlib import ExitStack

import concourse.bass as bass
import concourse.bacc as bacc
import concourse.tile as tile
import concourse.mybir as mybir
from concourse.bass_utils import run_bass_kernel_spmd

F32 = mybir.dt.float32
BF16 = mybir.dt.bfloat16
FP8 = mybir.dt.float8e4
I16 = mybir.dt.int16
BF16_NP = ml_dtypes.bfloat16

B, T, C = 4, 2048, 1024
H, D = 16, 64
HL = 8          # heads per core
N_CORES = 8
CC = C // 128   # 8 contraction chunks for QKV
TB = T // 512   # 4 token blocks of 512
TT = T // 128   # 16 token chunks of 128
EXPFN = mybir.ActivationFunctionType.Exp
DR = mybir.MatmulPerfMode.DoubleRow

USE_FP8_QK = True

# Schraudolph-style exp in bf16 bit space: i16 = x*SCHR_A + SCHR_B, then
# reinterpret the int16 as bf16.  SCHR_A folds the 1/sqrt(D) score scale.
SCHR_A = 0.125 * 128.0 / float(np.log(2.0))
SCHR_B = 127.0 * 128.0 - 7.41

# exp engine per (kp, j2, kc-half) slot (32 tiles of [128,512] per
# block): 'A' = ScalarE native exp, 'V' = DVE bitwise fast-exp.  GPSIMD
# cannot read PSUM, so only these two engines can consume score tiles.
# Pairs per (kp, j2) stay on one engine so the 4-deep PSUM ring
# alternates engines every two slots.  The first three blocks are
# PE-bound (projections interleave there), so they lean on the
# otherwise-idle DVE; engine-bound mid blocks lean on ScalarE, whose
# per-row exp is cheaper.


def _exp_pattern(a_pairs):
    pat = []
    acc = 0
    for i in range(16):
        take_a = (i * a_pairs) // 16 != ((i + 1) * a_pairs) // 16
        pat += ['A', 'A'] if take_a else ['V', 'V']
        acc += take_a
    return pat


EXP_EARLY = _exp_pattern(7)    # blocks 0-2: 14 A / 18 V
EXP_MID_E = _exp_pattern(10)   # even mid blocks: 20 A / 12 V
EXP_MID_O = _exp_pattern(9)    # odd mid blocks: 18 A / 14 V


def build_program():
    nc = bacc.Bacc("TRN2", debug=False, num_devices=1, target_bir_lowering=False)

    xT = nc.dram_tensor("xT", [C, T], BF16, kind="ExternalInput").ap()
    wqT = nc.dram_tensor("wqT", [C, 512], BF16, kind="ExternalInput").ap()
    wkT = nc.dram_tensor("wkT", [C, 512], BF16, kind="ExternalInput").ap()
    wvT = nc.dram_tensor("wvT", [C, 512], BF16, kind="ExternalInput").ap()
    bqk = nc.dram_tensor("bqk", [128, 8], F32, kind="ExternalInput").ap()
    bvb = nc.dram_tensor("bvb", [128, 512], F32, kind="ExternalInput").ap()
    wpT = nc.dram_tensor("wpT", [4, 128, 1024], BF16, kind="ExternalInput").ap()
    ident = nc.dram_tensor("ident", [128, 128], BF16, kind="ExternalInput").ap()
    yp = nc.dram_tensor("yp", [T, C], F32, kind="ExternalOutput").ap()

    qk_dt = FP8 if USE_FP8_QK else BF16

    with tile.TileContext(nc) as tc, ExitStack() as top:
        cpool = top.enter_context(tc.tile_pool(name="consts", bufs=1))
        bqk_sb = cpool.tile([128, 8], F32, tag="bqk")
        bvb_sb = cpool.tile([128, 512], F32, tag="bvb")
        id_sb = cpool.tile([128, 128], BF16, tag="ident")

        actpool = top.enter_context(tc.tile_pool(name="acts", bufs=1))
        # OT: attention output, feature-major [feat 128, tok 512] bf16
        OT = {(g, qb): actpool.tile([128, 512], BF16, tag=f"ot{g}_{qb}",
                                    name=f"ot{g}_{qb}")
              for g in range(4) for qb in range(4)}
        # Q/K in DoubleRow layout: [128 feat(2 heads), 2 k-subtiles, 2048 tok]
        # subtile 1 is zeros (fp8) / unused (bf16).
        QDR = {g: actpool.tile([128, 2, T], qk_dt, tag=f"qdr{g}",
                               name=f"qdr{g}") for g in range(4)}
        KDR = {g: actpool.tile([128, 2, T], qk_dt, tag=f"kdr{g}",
                               name=f"kdr{g}") for g in range(4)}
        V = [actpool.tile([128, HL * 65], BF16, tag=f"v{tt}", name=f"v{tt}")
             for tt in range(TT)]

        if USE_FP8_QK:
            # zero the second k-subtile once (matmul contracts over both)
            for g in range(4):
                nc.gpsimd.memset(QDR[g][:, 1, :], 0.0)
                nc.gpsimd.memset(KDR[g][:, 1, :], 0.0)

        ps1cm = tc.tile_pool(name="ps1", bufs=2, space="PSUM")
        ps1pool = ps1cm.__enter__()
        attncm = [tc.tile_pool(name="pt", bufs=1),
                  tc.tile_pool(name="ps2", bufs=4, space="PSUM"),
                  tc.tile_pool(name="pvtr", bufs=2, space="PSUM"),
                  tc.tile_pool(name="rr", bufs=2),
                  tc.tile_pool(name="otm", bufs=2)]
        (ptpool, ps2pool, pvpool, rrpool, otmpool) = \
            [cm.__enter__() for cm in attncm]

        xbcm = tc.tile_pool(name="xball", bufs=1)
        xbpool = xbcm.__enter__()

        # x feature-major, all of it: [128, cc, tok]; loaded in 4 big DMAs
        xar = xbpool.tile([128, CC, T], BF16, tag="xar")
        xTr = xT.rearrange("(c p) t -> p c t", p=128)
        wq_all = xbpool.tile([128, CC * 512], BF16, tag="wq_all")
        wqr = wqT.rearrange("(c p) j -> p c j", p=128)
        wqv = wq_all[:].rearrange("p (c j) -> p c j", j=512)
        wk_all = xbpool.tile([128, CC * 512], BF16, tag="wk_all")
        nc.sync.dma_start(wqv[:, 0:4], wqr[:, 0:4])
        nc.sync.dma_start(xar[:, :, 0:512], xTr[:, :, 0:512])
        nc.sync.dma_start(wqv[:, 4:8], wqr[:, 4:8])
        nc.sync.dma_start(bqk_sb[:], bqk[:])
        nc.sync.dma_start(
            wk_all[:].rearrange("p (c j) -> p c j", j=512),
            wkT.rearrange("(c p) j -> p c j", p=128))
        for tb in range(1, TB):
            nc.sync.dma_start(xar[:, :, tb * 512:(tb + 1) * 512],
                              xTr[:, :, tb * 512:(tb + 1) * 512])
        wv_all = xbpool.tile([128, CC, 512], BF16, tag="wv_all")
        nc.sync.dma_start(wv_all[:],
                          wvT.rearrange("(c p) j -> p c j", p=128))
        nc.sync.dma_start(bvb_sb[:], bvb[:])
        nc.sync.dma_start(id_sb[:], ident[:])

        def xb(tb, cc):
            return xar[:, cc, tb * 512:(tb + 1) * 512]

        def qk_proj_piece(g, tb, which):
            """One Q or K projection group for head pair g, token block tb.
            The bias add runs on ScalarE (per-partition bias AP is legal
            there) and quantizes to fp8 on the way out."""
            w_all, dst, bcol = ((wq_all, QDR[g], g) if which == 0 else
                                (wk_all, KDR[g], 4 + g))
            ps = ps1pool.tile([128, 512], F32, tag="ps1", name="psqk")
            for cc in range(CC):
                co = cc * 512 + g * 128
                nc.tensor.matmul(
                    ps[:], w_all[:, co:co + 128],
                    xb(tb, cc),
                    start=(cc == 0), stop=(cc == CC - 1))
            nc.scalar.activation(
                dst[:, 0, tb * 512:(tb + 1) * 512], ps[:],
                mybir.ActivationFunctionType.Identity,
                bias=bqk_sb[:, bcol:bcol + 1])

        def qk_proj(g):
            for tb in range(TB):
                for which in range(2):
                    qk_proj_piece(g, tb, which)

        def v_proj_piece(tt):
            ps = ps1pool.tile([128, 512], F32, tag="ps1", name="psv")
            for cc in range(CC):
                nc.tensor.matmul(
                    ps[:], xar[:, cc, tt * 128:(tt + 1) * 128],
                    wv_all[:, cc, :],
                    start=(cc == 0), stop=(cc == CC - 1))
            v3 = V[tt][:].rearrange("p (h x) -> p h x", x=65)
            nc.gpsimd.memset(v3[:, :, 64:65], 1.0)
            nc.vector.scalar_tensor_tensor(
                v3[:, :, 0:64],
                ps[:].rearrange("p (h x) -> p h x", x=64), 1.0,
                bvb_sb[:].rearrange("p (h x) -> p h x", x=64),
                op0=mybir.AluOpType.mult, op1=mybir.AluOpType.add)

        # ---- attention blocks, software-pipelined -----------------------
        # block n = (g, qb), qb-major so every fourth block finishes an
        # OT column and the output projection can interleave early.
        # scores(n) and PV(n-1) interleave in the PE stream so PE has
        # work while exp drains score PSUMs.
        blocks = [(g, qb) for qb in range(4) for g in range(4)]
        pts = {}      # (parity, kp, j2) -> exp'd score tile [128, 1024]
        pv_state = {}  # live PV psum tiles per j2

        def scores_mm(n, g, qb, kp, j2):
            """Scores for head 2g+j2, key chunks 2kp/2kp+1, queries qb:
            two [128, 512] tiles (one per key chunk), each exp'd as soon
            as it fills."""
            fo = 64 * j2
            for j in range(2):
                kc = 2 * kp + j
                pp = ps2pool.tile([128, 512], F32, tag="ps2",
                                  name=f"sc{j2}")
                if USE_FP8_QK:
                    for u in range(2):
                        nc.tensor.matmul(
                            pp[:, u * 256:(u + 1) * 256],
                            KDR[g][fo:fo + 64, :, kc * 128:(kc + 1) * 128],
                            QDR[g][fo:fo + 64, :,
                                   qb * 512 + u * 256:qb * 512 + (u + 1) * 256],
                            start=True, stop=True, perf_mode=DR)
                else:
                    nc.tensor.matmul(
                        pp[:],
                        KDR[g][fo:fo + 64, 0, kc * 128:(kc + 1) * 128],
                        QDR[g][fo:fo + 64, 0, qb * 512:(qb + 1) * 512],
                        start=True, stop=True)
                pt = ptpool.tile([128, 512], BF16,
                                 tag=f"pt{n % 2}_{kc}_{j2}",
                                 name=f"pt{n % 2}_{kc}_{j2}")
                if EXP_ENG[4 * kp + 2 * j2 + j] == 'A':
                    nc.scalar.activation(pt[:], pp[:], EXPFN, scale=0.125)
                else:
                    nc.vector.tensor_scalar(pt[:].bitcast(I16), pp[:],
                                            SCHR_A, SCHR_B,
                                            op0=mybir.AluOpType.mult,
                                            op1=mybir.AluOpType.add)
                pts[(n % 2, kc, j2)] = pt

        # PV accumulation order per head half: groups ts0..ts3, 16 key
        # chunks each, strictly sequential (one open accumulation group
        # per PSUM bank).  Spread over steps 0..5 so the finish work can
        # run at steps 6-7 without delaying the next block's exps.
        PV_SPLIT = [0, 11, 22, 33, 44, 54, 64]

        def pv_mm(n, g, qb, step):
            par = n % 2
            for j2 in range(2):
                h = 2 * g + j2
                if step == 0:
                    pv_state[j2] = pvpool.tile([128, 260], F32,
                                               tag="pvtr", name=f"pv{j2}")
                pv = pv_state[j2]
                for i in range(PV_SPLIT[step], PV_SPLIT[step + 1]):
                    ts, kc = i // 16, i % 16
                    nc.tensor.matmul(
                        pv[:, ts * 65:(ts + 1) * 65],
                        pts[(par, kc, j2)][:, ts * 128:(ts + 1) * 128],
                        V[kc][:, h * 65:(h + 1) * 65],
                        start=(kc == 0), stop=(kc == 15))

        otm_state = {}

        def pv_finish_a(n, g, qb):
            """Normalize block n's PV accumulators (runs at step 6):
            reciprocal on DVE, the per-token scale on ScalarE."""
            for j2 in range(2):
                pv = pv_state.pop(j2)
                rr = rrpool.tile([128, 4], F32, tag="rr", name="rr")
                pv3 = pv[:].rearrange("p (t x) -> p t x", x=65)
                nc.vector.reciprocal(rr[:], pv3[:, :, 64])
                ot = otmpool.tile([128, 256], BF16, tag=f"otm{j2}",
                                  name=f"otm{j2}")
                nc.vector.tensor_tensor(
                    ot[:].rearrange("p (t x) -> p t x", x=64),
                    pv3[:, :, 0:64],
                    rr[:].unsqueeze(-1).broadcast_to([128, 4, 64]),
                    op=mybir.AluOpType.mult)
                otm_state[j2] = ot

        def pv_finish_b(n, g, qb):
            """Transpose + store OT tiles for block n (runs at step 7)."""
            otm = {j2: otm_state.pop(j2) for j2 in range(2)}
            for ts in range(4):
                tr = ps1pool.tile([128, 128], BF16, tag="ps1", name="tr")
                for j2 in range(2):
                    nc.tensor.matmul(
                        tr[64 * j2:64 * j2 + 64, :],
                        otm[j2][:, ts * 64:(ts + 1) * 64],
                        id_sb[:], start=True, stop=True, is_transpose=True)
                nc.vector.tensor_copy(
                    OT[(g, qb)][:, ts * 128:(ts + 1) * 128], tr[:])

        # ---- output projection piece (interleaved into late blocks) ----
        wp_state = {}

        def outproj_load():
            wpcm = tc.tile_pool(name="wp", bufs=1)
            ycm = tc.tile_pool(name="ysb", bufs=2)
            wp_state["cms"] = [wpcm, ycm]
            wppool = wpcm.__enter__()
            ypool = ycm.__enter__()
            wp_all = wppool.tile([128, 4096], BF16, tag="wp_all")
            nc.sync.dma_start(
                wp_all[:].rearrange("p (j o) -> p j o", o=1024),
                wpT.rearrange("j p o -> p j o"))
            wp_state.update(wp_all=wp_all, ypool=ypool)

        def outproj_piece(tt):
            # b_proj is added on the host; this is a plain PSUM drain,
            # split across ScalarE and DVE.
            wp_all = wp_state["wp_all"]
            y_sb = wp_state["ypool"].tile([128, 1024], F32, tag="y",
                                          name="y_sb")
            for cb in range(2):
                ps = ps1pool.tile([128, 512], F32, tag="ps1", name="psy")
                for j in range(4):
                    nc.tensor.matmul(
                        ps[:],
                        OT[(j, tt // 4)][:, (tt % 4) * 128:
                                         (tt % 4 + 1) * 128],
                        wp_all[:, j * 1024 + cb * 512:
                               j * 1024 + (cb + 1) * 512],
                        start=(j == 0), stop=(j == 3))
                if cb == 0:
                    nc.scalar.copy(y_sb[:, 0:512], ps[:])
                else:
                    nc.vector.tensor_copy(y_sb[:, 512:1024], ps[:])
            nc.sync.dma_start(yp[tt * 128:(tt + 1) * 128, :], y_sb[:])

        # ---- emit ------------------------------------------------------
        # outproj tile tt is ready once OT[(3, tt//4)] exists, i.e. after
        # pv_finish_b(block 4*(tt//4)+3) which is emitted during block
        # 4*(tt//4)+4; schedule one tile per block starting one later so
        # PE-light blocks all carry some slack work.
        outmap = {5: [0], 6: [1], 7: [2], 8: [3], 9: [4], 10: [5],
                  11: [6], 12: [7], 13: [8], 14: [9], 15: [10, 11]}
        # startup: only the q/k tiles the first scores need; the rest of
        # projection 0, V, and projection 1 interleave into block 0 in
        # deadline order (K chunk tb feeds scores step 2*tb; V feeds
        # block 1's PV; projection 1 feeds block 1).
        qk_proj_piece(0, 0, 0)
        qk_proj_piece(0, 0, 1)
        for n, (g, qb) in enumerate(blocks):
            if n == 4:
                # x / qkv-weight tiles are dead; reuse their SBUF for the
                # output projection weights
                xbcm.__exit__(None, None, None)
                outproj_load()
            extra = []
            if n == 0:
                extra = [(qk_proj_piece, (0, tb, 1)) for tb in (1, 2, 3)]
                extra += [(v_proj_piece, (tt,)) for tt in range(8)]
                extra += [(qk_proj_piece, (1, 0, w)) for w in range(2)]
                extra += [(v_proj_piece, (tt,)) for tt in range(8, TT)]
                extra += [(qk_proj_piece, (1, tb, w))
                          for tb in (1, 2, 3) for w in range(2)]
                extra += [(qk_proj_piece, (0, tb, 0)) for tb in (1, 2, 3)]
            elif n < 3:
                extra = [(qk_proj_piece, (n + 1, tb, w))
                         for tb in range(TB) for w in range(2)]
            extra += [(outproj_piece, (tt,)) for tt in outmap.get(n, [])]
            npc = (len(extra) + 7) // 8  # extra pieces per step
            for kp in range(8):
                if n > 0:
                    if kp < 6:
                        pv_mm(n - 1, *blocks[n - 1], step=kp)
                    elif kp == 6:
                        pv_finish_a(n - 1, *blocks[n - 1])
                    else:
                        pv_finish_b(n - 1, *blocks[n - 1])
                if n == 0:
                    scores_mm(n, g, qb, kp, 0)
                    scores_mm(n, g, qb, kp, 1)
                for fn, args in extra[kp * npc:(kp + 1) * npc]:
                    fn(*args)
                if n > 0:
                    scores_mm(n, g, qb, kp, 0)
                    scores_mm(n, g, qb, kp, 1)
        for kp in range(6):
            pv_mm(15, *blocks[15], step=kp)
        pv_finish_a(15, *blocks[15])
        pv_finish_b(15, *blocks[15])
        for tt in range(12, 16):
            outproj_piece(tt)

        for cm in reversed(wp_state["cms"]):
            cm.__exit__(None, None, None)
        for cm in reversed(attncm):
            cm.__exit__(None, None, None)
        ps1cm.__exit__(None, None, None)

    nc.compile()
    return nc


_NC_CACHE = None


def get_program():
    global _NC_CACHE
    if _NC_CACHE is None:
        _NC_CACHE = build_program()
    return _NC_CACHE


def make_in_maps(x, w_qkv, b_qkv, w_proj, b_proj):
    x = np.asarray(x, dtype=np.float32)
    w_qkv = np.asarray(w_qkv, dtype=np.float32)
    b_qkv = np.asarray(b_qkv, dtype=np.float32)
    w_proj = np.asarray(w_proj, dtype=np.float32)
    b_proj = np.asarray(b_proj, dtype=np.float32)

    xTs = [np.ascontiguousarray(x[b].T).astype(BF16_NP) for b in range(B)]
    ident = np.eye(128, dtype=np.float32).astype(BF16_NP)

    grp = []
    for hg in range(2):
        sl = slice(hg * 512, (hg + 1) * 512)
        wq = w_qkv[0:C][sl]
        wk = w_qkv[C:2 * C][sl]
        wv = w_qkv[2 * C:3 * C][sl]
        bq = b_qkv[0:C][sl]
        bk = b_qkv[C:2 * C][sl]
        bv = b_qkv[2 * C:3 * C][sl]
        grp.append(dict(
            wqT=np.ascontiguousarray(wq.T).astype(BF16_NP),
            wkT=np.ascontiguousarray(wk.T).astype(BF16_NP),
            wvT=np.ascontiguousarray(wv.T).astype(BF16_NP),
            bqk=np.stack([bq[i * 128:(i + 1) * 128] for i in range(4)]
                         + [bk[i * 128:(i + 1) * 128] for i in range(4)],
                         axis=1).astype(np.float32),
            bvb=np.tile(bv[None, :], (128, 1)).astype(np.float32),
            wpT=np.ascontiguousarray(
                w_proj[:, sl].T).reshape(4, 128, 1024).astype(BF16_NP),
            ident=ident,
        ))

    in_maps = []
    for core in range(N_CORES):
        b, hg = core // 2, core % 2
        m = {"xT": xTs[b]}
        m.update(grp[hg])
        in_maps.append(m)
    return in_maps


def kernel(x, w_qkv, b_qkv, w_proj, b_proj):
    nc = get_program()
    in_maps = make_in_maps(x, w_qkv, b_qkv, w_proj, b_proj)
    res = run_bass_kernel_spmd(
        nc, in_maps, core_ids=list(range(N_CORES)), trace=False)
    bp = np.asarray(b_proj, dtype=np.float32)
    y = np.empty((B, T, C), dtype=np.float32)
    for b in range(B):
        y[b] = res.results[2 * b]["yp"] + res.results[2 * b + 1]["yp"] + bp
    return y


# revision 77
# speedup vs baseline: 1.1850x; 1.0051x over previous
"""Multi-head self-attention Trainium2 kernel (B=4, T=2048, C=1024, H=16, D=64).

Sharding: 8 cores = 4 batches x 2 head-groups (8 heads each). Each core
computes its batch's QKV for its heads, attention, and a partial output
projection (row-sharded over attention features). The host sums the two
partials per batch (each partial carries b_proj/2, so the pair sums to
b_proj exactly).

v2 optimizations over the baseline:
  - Scores run as fp8e4 DoubleRow matmuls (0.5 cycles/row): Q/K are
    quantized to fp8 by the bias-add, stored [128, 2, 2048] with the
    second k-subtile zeroed (D=64 < 128, so the pair is (dims, zeros)).
  - P@V runs transposed: out[tok, dim] with P as stationary
    ([128 keys, 128 tok] slices) and V[keys, 65] as moving (ones column
    produces the softmax denominator), so each matmul moves 65 elements
    instead of 512.
  - Softmax normalization becomes a per-partition scalar multiply, then
    PE transposes restore [feat, tok] tiles for the output projection.
  - exp is split across ScalarE (native) and DVE/Pool (bitwise fast-exp
    into bf16 bits).
"""

import numpy as np
import ml_dtypes
from contextlib import ExitStack

import concourse.bass as bass
import concourse.bacc as bacc
import concourse.tile as tile
import concourse.mybir as mybir
from concourse.bass_utils import run_bass_kernel_spmd

F32 = mybir.dt.float32
BF16 = mybir.dt.bfloat16
FP8 = mybir.dt.float8e4
I16 = mybir.dt.int16
BF16_NP = ml_dtypes.bfloat16

B, T, C = 4, 2048, 1024
H, D = 16, 64
HL = 8          # heads per core
N_CORES = 8
CC = C // 128   # 8 contraction chunks for QKV
TB = T // 512   # 4 token blocks of 512
TT = T // 128   # 16 token chunks of 128
EXPFN = mybir.ActivationFunctionType.Exp
DR = mybir.MatmulPerfMode.DoubleRow

USE_FP8_QK = True

# Schraudolph-style exp in bf16 bit space: i16 = x*SCHR_A + SCHR_B, then
# reinterpret the int16 as bf16.  SCHR_A folds the 1/sqrt(D) score scale.
SCHR_A = 0.125 * 128.0 / float(np.log(2.0))
SCHR_B = 127.0 * 128.0 - 7.41

# exp engine per (kp, j2, kc-half) slot (32 tiles of [128,512] per
# block): 'A' = ScalarE native exp, 'V' = DVE bitwise fast-exp.  GPSIMD
# cannot read PSUM, so only these two engines can consume score tiles.
# Pairs per (kp, j2) stay on one engine so the 4-deep PSUM ring
# alternates engines every two slots.  The first three blocks are
# PE-bound (projections interleave there), so they lean on the
# otherwise-idle DVE; engine-bound mid blocks lean on ScalarE, whose
# per-row exp is cheaper.


EXP_MID_E = ['A', 'V'] * 14 + ['A'] * 4      # 18 A / 14 V
EXP_EARLY = EXP_MID_E
EXP_MID_O = EXP_MID_E


def build_program():
    nc = bacc.Bacc("TRN2", debug=False, num_devices=1, target_bir_lowering=False)

    xT = nc.dram_tensor("xT", [C, T], BF16, kind="ExternalInput").ap()
    wqT = nc.dram_tensor("wqT", [C, 512], BF16, kind="ExternalInput").ap()
    wkT = nc.dram_tensor("wkT", [C, 512], BF16, kind="ExternalInput").ap()
    wvT = nc.dram_tensor("wvT", [C, 512], BF16, kind="ExternalInput").ap()
    bqk = nc.dram_tensor("bqk", [128, 8], F32, kind="ExternalInput").ap()
    wpT = nc.dram_tensor("wpT", [4, 128, 1024], BF16, kind="ExternalInput").ap()
    ident = nc.dram_tensor("ident", [128, 128], BF16, kind="ExternalInput").ap()
    yp = nc.dram_tensor("yp", [T, C], F32, kind="ExternalOutput").ap()

    qk_dt = FP8 if USE_FP8_QK else BF16

    with tile.TileContext(nc) as tc, ExitStack() as top:
        cpool = top.enter_context(tc.tile_pool(name="consts", bufs=1))
        bqk_sb = cpool.tile([128, 8], F32, tag="bqk")
        id_sb = cpool.tile([128, 128], BF16, tag="ident")

        actpool = top.enter_context(tc.tile_pool(name="acts", bufs=1))
        # OT: attention output, feature-major [feat 128, tok 512] bf16
        OT = {(g, qb): actpool.tile([128, 512], BF16, tag=f"ot{g}_{qb}",
                                    name=f"ot{g}_{qb}")
              for g in range(4) for qb in range(4)}
        # Q/K in DoubleRow layout: [128 feat(2 heads), 2 k-subtiles, 2048 tok]
        # subtile 1 is zeros (fp8) / unused (bf16).
        QDR = {g: actpool.tile([128, 2, T], qk_dt, tag=f"qdr{g}",
                               name=f"qdr{g}") for g in range(4)}
        KDR = {g: actpool.tile([128, 2, T], qk_dt, tag=f"kdr{g}",
                               name=f"kdr{g}") for g in range(4)}
        V = [actpool.tile([128, HL * 65], BF16, tag=f"v{tt}", name=f"v{tt}")
             for tt in range(TT)]

        if USE_FP8_QK:
            # zero the second k-subtile once (matmul contracts over both)
            for g in range(4):
                nc.gpsimd.memset(QDR[g][:, 1, :], 0.0)
                nc.gpsimd.memset(KDR[g][:, 1, :], 0.0)

        ps1cm = tc.tile_pool(name="ps1", bufs=2, space="PSUM")
        ps1pool = ps1cm.__enter__()
        attncm = [tc.tile_pool(name="pt", bufs=1),
                  tc.tile_pool(name="ps2", bufs=4, space="PSUM"),
                  tc.tile_pool(name="pvtr", bufs=2, space="PSUM"),
                  tc.tile_pool(name="rr", bufs=2),
                  tc.tile_pool(name="otm", bufs=2)]
        (ptpool, ps2pool, pvpool, rrpool, otmpool) = \
            [cm.__enter__() for cm in attncm]

        xbcm = tc.tile_pool(name="xball", bufs=1)
        xbpool = xbcm.__enter__()

        # x feature-major, all of it: [128, cc, tok]; loaded in 4 big DMAs
        xar = xbpool.tile([128, CC, T], BF16, tag="xar")
        xTr = xT.rearrange("(c p) t -> p c t", p=128)
        wq_all = xbpool.tile([128, CC * 512], BF16, tag="wq_all")
        wqr = wqT.rearrange("(c p) j -> p c j", p=128)
        wqv = wq_all[:].rearrange("p (c j) -> p c j", j=512)
        wk_all = xbpool.tile([128, CC * 512], BF16, tag="wk_all")
        nc.sync.dma_start(wqv[:, 0:4], wqr[:, 0:4])
        nc.sync.dma_start(xar[:, :, 0:512], xTr[:, :, 0:512])
        nc.sync.dma_start(wqv[:, 4:8], wqr[:, 4:8])
        nc.sync.dma_start(bqk_sb[:], bqk[:])
        nc.sync.dma_start(
            wk_all[:].rearrange("p (c j) -> p c j", j=512),
            wkT.rearrange("(c p) j -> p c j", p=128))
        for tb in range(1, TB):
            nc.sync.dma_start(xar[:, :, tb * 512:(tb + 1) * 512],
                              xTr[:, :, tb * 512:(tb + 1) * 512])
        wv_all = xbpool.tile([128, CC, 512], BF16, tag="wv_all")
        nc.sync.dma_start(wv_all[:],
                          wvT.rearrange("(c p) j -> p c j", p=128))
        nc.sync.dma_start(id_sb[:], ident[:])

        def xb(tb, cc):
            return xar[:, cc, tb * 512:(tb + 1) * 512]

        def qk_proj_piece(g, tb, which):
            """One Q or K projection group for head pair g, token block tb.
            The bias add runs on ScalarE (per-partition bias AP is legal
            there) and quantizes to fp8 on the way out."""
            w_all, dst, bcol = ((wq_all, QDR[g], g) if which == 0 else
                                (wk_all, KDR[g], 4 + g))
            ps = ps1pool.tile([128, 512], F32, tag="ps1", name="psqk")
            for cc in range(CC):
                co = cc * 512 + g * 128
                nc.tensor.matmul(
                    ps[:], w_all[:, co:co + 128],
                    xb(tb, cc),
                    start=(cc == 0), stop=(cc == CC - 1))
            nc.scalar.activation(
                dst[:, 0, tb * 512:(tb + 1) * 512], ps[:],
                mybir.ActivationFunctionType.Identity,
                bias=bqk_sb[:, bcol:bcol + 1])

        def qk_proj(g):
            for tb in range(TB):
                for which in range(2):
                    qk_proj_piece(g, tb, which)

        def v_proj_piece(tt):
            ps = ps1pool.tile([128, 512], F32, tag="ps1", name="psv")
            for cc in range(CC):
                nc.tensor.matmul(
                    ps[:], xar[:, cc, tt * 128:(tt + 1) * 128],
                    wv_all[:, cc, :],
                    start=(cc == 0), stop=(cc == CC - 1))
            # V carries no bias: softmax weights sum to 1, so the bias
            # contribution is bv @ Wp per token, folded into the host-side
            # b_proj add.
            v3 = V[tt][:].rearrange("p (h x) -> p h x", x=65)
            nc.gpsimd.memset(v3[:, :, 64:65], 1.0)
            src = ps[:].rearrange("p (h x) -> p h x", x=64)
            nc.vector.tensor_copy(v3[:, :, 0:64], src)

        # ---- attention blocks, software-pipelined -----------------------
        # block n = (g, qb), qb-major so every fourth block finishes an
        # OT column and the output projection can interleave early.
        # scores(n) and PV(n-1) interleave in the PE stream so PE has
        # work while exp drains score PSUMs.
        blocks = [(g, qb) for qb in range(4) for g in range(4)]
        pts = {}      # (parity, kp, j2) -> exp'd score tile [128, 1024]
        pv_state = {}  # live PV psum tiles per j2

        def scores_mm(n, g, qb, kp, j2):
            """Scores for head 2g+j2, key chunks 2kp/2kp+1, queries qb:
            two [128, 512] tiles (one per key chunk), each exp'd as soon
            as it fills."""
            fo = 64 * j2
            for j in range(2):
                kc = 2 * kp + j
                pp = ps2pool.tile([128, 512], F32, tag="ps2",
                                  name=f"sc{j2}")
                if USE_FP8_QK:
                    for u in range(2):
                        nc.tensor.matmul(
                            pp[:, u * 256:(u + 1) * 256],
                            KDR[g][fo:fo + 64, :, kc * 128:(kc + 1) * 128],
                            QDR[g][fo:fo + 64, :,
                                   qb * 512 + u * 256:qb * 512 + (u + 1) * 256],
                            start=True, stop=True, perf_mode=DR)
                else:
                    nc.tensor.matmul(
                        pp[:],
                        KDR[g][fo:fo + 64, 0, kc * 128:(kc + 1) * 128],
                        QDR[g][fo:fo + 64, 0, qb * 512:(qb + 1) * 512],
                        start=True, stop=True)
                pt = ptpool.tile([128, 512], BF16,
                                 tag=f"pt{n % 2}_{kc}_{j2}",
                                 name=f"pt{n % 2}_{kc}_{j2}")
                pat = (EXP_EARLY if n < 3 else
                       EXP_MID_E if n % 2 == 0 else EXP_MID_O)
                if pat[4 * kp + 2 * j2 + j] == 'A':
                    nc.scalar.activation(pt[:], pp[:], EXPFN, scale=0.125)
                else:
                    nc.vector.tensor_scalar(pt[:].bitcast(I16), pp[:],
                                            SCHR_A, SCHR_B,
                                            op0=mybir.AluOpType.mult,
                                            op1=mybir.AluOpType.add)
                pts[(n % 2, kc, j2)] = pt

        # PV accumulation order per head half: groups ts0..ts3, 16 key
        # chunks each, strictly sequential (one open accumulation group
        # per PSUM bank).  Spread over steps 0..5 so the finish work can
        # run at steps 6-7 without delaying the next block's exps.
        PV_SPLIT = [0, 11, 22, 33, 44, 54, 64]

        def pv_mm(n, g, qb, step):
            par = n % 2
            for j2 in range(2):
                h = 2 * g + j2
                if step == 0:
                    pv_state[j2] = pvpool.tile([128, 260], F32,
                                               tag="pvtr", name=f"pv{j2}")
                pv = pv_state[j2]
                for i in range(PV_SPLIT[step], PV_SPLIT[step + 1]):
                    ts, kc = i // 16, i % 16
                    nc.tensor.matmul(
                        pv[:, ts * 65:(ts + 1) * 65],
                        pts[(par, kc, j2)][:, ts * 128:(ts + 1) * 128],
                        V[kc][:, h * 65:(h + 1) * 65],
                        start=(kc == 0), stop=(kc == 15))

        otm_state = {}

        def pv_finish_a(n, g, qb):
            """Normalize block n's PV accumulators (runs at step 6):
            reciprocal on DVE, the per-token scale on ScalarE."""
            for j2 in range(2):
                pv = pv_state.pop(j2)
                rr = rrpool.tile([128, 4], F32, tag="rr", name="rr")
                pv3 = pv[:].rearrange("p (t x) -> p t x", x=65)
                nc.vector.reciprocal(rr[:], pv3[:, :, 64])
                ot = otmpool.tile([128, 256], BF16, tag=f"otm{j2}",
                                  name=f"otm{j2}")
                nc.vector.tensor_tensor(
                    ot[:].rearrange("p (t x) -> p t x", x=64),
                    pv3[:, :, 0:64],
                    rr[:].unsqueeze(-1).broadcast_to([128, 4, 64]),
                    op=mybir.AluOpType.mult)
                otm_state[j2] = ot

        def pv_finish_b(n, g, qb):
            """Transpose + store OT tiles for block n (runs at step 7)."""
            otm = {j2: otm_state.pop(j2) for j2 in range(2)}
            for ts in range(4):
                tr = ps1pool.tile([128, 128], BF16, tag="ps1", name="tr")
                for j2 in range(2):
                    nc.tensor.matmul(
                        tr[64 * j2:64 * j2 + 64, :],
                        otm[j2][:, ts * 64:(ts + 1) * 64],
                        id_sb[:], start=True, stop=True, is_transpose=True)
                nc.vector.tensor_copy(
                    OT[(g, qb)][:, ts * 128:(ts + 1) * 128], tr[:])

        # ---- output projection piece (interleaved into late blocks) ----
        wp_state = {}

        def outproj_load():
            wpcm = tc.tile_pool(name="wp", bufs=1)
            ycm = tc.tile_pool(name="ysb", bufs=2)
            wp_state["cms"] = [wpcm, ycm]
            wppool = wpcm.__enter__()
            ypool = ycm.__enter__()
            wp_all = wppool.tile([128, 4096], BF16, tag="wp_all")
            nc.sync.dma_start(
                wp_all[:].rearrange("p (j o) -> p j o", o=1024),
                wpT.rearrange("j p o -> p j o"))
            wp_state.update(wp_all=wp_all, ypool=ypool)

        def outproj_piece(tt):
            # b_proj is added on the host; this is a plain PSUM drain,
            # split across ScalarE and DVE.
            wp_all = wp_state["wp_all"]
            y_sb = wp_state["ypool"].tile([128, 1024], F32, tag="y",
                                          name="y_sb")
            for cb in range(2):
                ps = ps1pool.tile([128, 512], F32, tag="ps1", name="psy")
                for j in range(4):
                    nc.tensor.matmul(
                        ps[:],
                        OT[(j, tt // 4)][:, (tt % 4) * 128:
                                         (tt % 4 + 1) * 128],
                        wp_all[:, j * 1024 + cb * 512:
                               j * 1024 + (cb + 1) * 512],
                        start=(j == 0), stop=(j == 3))
                if cb == 0:
                    nc.scalar.copy(y_sb[:, 0:512], ps[:])
                else:
                    nc.vector.tensor_copy(y_sb[:, 512:1024], ps[:])
                nc.sync.dma_start(
                    yp[tt * 128:(tt + 1) * 128, cb * 512:(cb + 1) * 512],
                    y_sb[:, cb * 512:(cb + 1) * 512])

        # ---- emit ------------------------------------------------------
        # outproj tile tt is ready once OT[(3, tt//4)] exists, i.e. after
        # pv_finish_b(block 4*(tt//4)+3) which is emitted during block
        # 4*(tt//4)+4; schedule one tile per block starting one later so
        # PE-light blocks all carry some slack work.
        outmap = {5: [0], 6: [1], 7: [2], 8: [3], 9: [4], 10: [5],
                  11: [6], 12: [7], 13: [8], 14: [9], 15: [10, 11]}
        # startup: only the q/k tiles the first scores need; the rest of
        # projection 0, V, and projection 1 interleave into block 0 in
        # deadline order (K chunk tb feeds scores step 2*tb; V feeds
        # block 1's PV; projection 1 feeds block 1).
        qk_proj_piece(0, 0, 0)
        qk_proj_piece(0, 0, 1)
        for n, (g, qb) in enumerate(blocks):
            if n == 4:
                # x / qkv-weight tiles are dead; reuse their SBUF for the
                # output projection weights
                xbcm.__exit__(None, None, None)
                outproj_load()
            extra = []
            if n == 0:
                extra = [(qk_proj_piece, (0, tb, 1)) for tb in (1, 2, 3)]
                extra += [(v_proj_piece, (tt,)) for tt in range(8)]
                extra += [(qk_proj_piece, (1, 0, w)) for w in range(2)]
                extra += [(v_proj_piece, (tt,)) for tt in range(8, TT)]
                extra += [(qk_proj_piece, (1, tb, w))
                          for tb in (1, 2, 3) for w in range(2)]
                extra += [(qk_proj_piece, (0, tb, 0)) for tb in (1, 2, 3)]
            elif n < 3:
                extra = [(qk_proj_piece, (n + 1, tb, w))
                         for tb in range(TB) for w in range(2)]
            extra += [(outproj_piece, (tt,)) for tt in outmap.get(n, [])]
            npc = (len(extra) + 7) // 8  # extra pieces per step
            for kp in range(8):
                if n > 0:
                    if kp < 6:
                        pv_mm(n - 1, *blocks[n - 1], step=kp)
                    elif kp == 6:
                        pv_finish_a(n - 1, *blocks[n - 1])
                    else:
                        pv_finish_b(n - 1, *blocks[n - 1])
                if n == 0:
                    scores_mm(n, g, qb, kp, 0)
                    scores_mm(n, g, qb, kp, 1)
                for fn, args in extra[kp * npc:(kp + 1) * npc]:
                    fn(*args)
                if n > 0:
                    scores_mm(n, g, qb, kp, 0)
                    scores_mm(n, g, qb, kp, 1)
        for kp in range(6):
            pv_mm(15, *blocks[15], step=kp)
        pv_finish_a(15, *blocks[15])
        pv_finish_b(15, *blocks[15])
        for tt in range(12, 16):
            outproj_piece(tt)

        for cm in reversed(wp_state["cms"]):
            cm.__exit__(None, None, None)
        for cm in reversed(attncm):
            cm.__exit__(None, None, None)
        ps1cm.__exit__(None, None, None)

    nc.compile()
    return nc


_NC_CACHE = None


def get_program():
    global _NC_CACHE
    if _NC_CACHE is None:
        _NC_CACHE = build_program()
    return _NC_CACHE


def make_in_maps(x, w_qkv, b_qkv, w_proj, b_proj):
    x = np.asarray(x, dtype=np.float32)
    w_qkv = np.asarray(w_qkv, dtype=np.float32)
    b_qkv = np.asarray(b_qkv, dtype=np.float32)
    w_proj = np.asarray(w_proj, dtype=np.float32)
    b_proj = np.asarray(b_proj, dtype=np.float32)

    xTs = [np.ascontiguousarray(x[b].T).astype(BF16_NP) for b in range(B)]
    ident = np.eye(128, dtype=np.float32).astype(BF16_NP)

    grp = []
    for hg in range(2):
        sl = slice(hg * 512, (hg + 1) * 512)
        wq = w_qkv[0:C][sl]
        wk = w_qkv[C:2 * C][sl]
        wv = w_qkv[2 * C:3 * C][sl]
        bq = b_qkv[0:C][sl]
        bk = b_qkv[C:2 * C][sl]
        bv = b_qkv[2 * C:3 * C][sl]
        grp.append(dict(
            wqT=np.ascontiguousarray(wq.T).astype(BF16_NP),
            wkT=np.ascontiguousarray(wk.T).astype(BF16_NP),
            wvT=np.ascontiguousarray(wv.T).astype(BF16_NP),
            bqk=np.stack([bq[i * 128:(i + 1) * 128] for i in range(4)]
                         + [bk[i * 128:(i + 1) * 128] for i in range(4)],
                         axis=1).astype(np.float32),
            wpT=np.ascontiguousarray(
                w_proj[:, sl].T).reshape(4, 128, 1024).astype(BF16_NP),
            ident=ident,
        ))

    in_maps = []
    for core in range(N_CORES):
        b, hg = core // 2, core % 2
        m = {"xT": xTs[b]}
        m.update(grp[hg])
        in_maps.append(m)
    return in_maps


def kernel(x, w_qkv, b_qkv, w_proj, b_proj):
    nc = get_program()
    in_maps = make_in_maps(x, w_qkv, b_qkv, w_proj, b_proj)
    res = run_bass_kernel_spmd(
        nc, in_maps, core_ids=list(range(N_CORES)), trace=False)
    # V's bias is not applied on-device: softmax weights sum to 1, so it
    # contributes bv @ w_proj.T per token, folded in here with b_proj.
    bp = (np.asarray(b_proj, dtype=np.float32)
          + np.asarray(b_qkv, dtype=np.float32)[2 * C:]
          @ np.asarray(w_proj, dtype=np.float32).T)
    y = np.empty((B, T, C), dtype=np.float32)
    for b in range(B):
        y[b] = res.results[2 * b]["yp"] + res.results[2 * b + 1]["yp"] + bp
    return y
